# revision 2
# baseline (speedup 1.0000x reference)
"""Trainium2 Bass kernel: 4x4-block 2D DCT over x[16, 64, 256, 256] fp32.

Math: for each 4x4 block B of each 256x256 image, out = D @ B @ D^T.
With R = kron(I_32, D^T) (128x128 block-diagonal), a [128(h), 128(w)] tile X
satisfies:  P1 = X^T @ R   (H-pass, transposed)
            P2 = P1^T @ R  (W-pass, final orientation [h', w'])
Both are single PE matmuls (out = lhsT.T @ rhs with lhsT = data, rhs = R),
so the per-pass transpose comes free from the matmul semantics.

Precision: the harness gate is rel_err < 2e-2; fp16 end-to-end measures
6.5e-4 (host-side numpy simulation vs fp64). fp16 halves HBM traffic
(33.5 MB -> per-core floor ~94 us at 358 GB/s) and runs the PE at
1 cyc/row instead of fp32's 4, so the kernel is DMA-bound as intended.

Sharding: pure data parallel - batch dim 16 -> 2 per core across 8 cores.
Per core: 128 images, processed as 16 supertiles (16 images x 1 h-chunk),
each supertile = one 1 MiB DMA in, 32 chained matmul pairs, one 1 MiB out.
"""

import numpy as np

import concourse.bass as bass
import concourse.mybir as mybir
import concourse.tile as tile
from concourse import bacc
from concourse.bass_utils import run_bass_kernel_spmd

N_CORES = 8
B_FULL, C, H, W = 16, 64, 256, 256
B_CORE = B_FULL // N_CORES          # 2 batches per core
IMGS = B_CORE * C                   # 128 images per core
HC = H // 128                       # h-chunks per image (2)
F32 = mybir.dt.float32
F16 = mybir.dt.float16

# "fp16": fp16 in/mid/out, fp32 PSUM accumulate (rel err ~6.5e-4, 2x less
# HBM traffic + 4x faster PE). "fp32": exact fp32 (rel err ~2.6e-7).
MODE = "fp16"


def _build_module(mode=MODE):
    fp16 = mode == "fp16"
    idt = F16 if fp16 else F32
    odt = F16 if fp16 else F32
    ipg = 16 if fp16 else 8                        # images per supertile

    nc = bacc.Bacc("TRN2", target_bir_lowering=False, debug=False,
                   num_devices=N_CORES)
    x_ap = nc.dram_tensor("x", [B_CORE, C, H, W], idt,
                          kind="ExternalInput").ap()
    r_ap = nc.dram_tensor("r", [128, 128], idt, kind="ExternalInput").ap()
    o_ap = nc.dram_tensor("out", [B_CORE, C, H, W], odt,
                          kind="ExternalOutput").ap()

    xi = x_ap.rearrange("b c h w -> (b c) h w")    # [128, 256, 256]
    oi = o_ap.rearrange("b c h w -> (b c) h w")
    nsub = ipg * W // 128                          # 128-col subtiles/supertile

    with tile.TileContext(nc) as tc:
        with (
            tc.tile_pool(name="const", bufs=1) as cpool,
            tc.tile_pool(name="xin", bufs=3) as xpool,
            tc.tile_pool(name="mid", bufs=3) as mpool,
            tc.tile_pool(name="oout", bufs=3) as opool,
            tc.tile_pool(name="ps1", bufs=4, space="PSUM") as p1pool,
            tc.tile_pool(name="ps2", bufs=4, space="PSUM") as p2pool,
        ):
            r_sb = cpool.tile([128, 128], idt)
            nc.sync.dma_start(out=r_sb[:], in_=r_ap[:])
            rhs = r_sb[:, 0:128]

            # Warm-up burst reading only r_sb: the first matmul absorbs the
            # r_sb DMA wait so no later matmul carries two semaphore waits
            # (Matmult supports at most one). The remaining back-to-back
            # matmuls give the PE ~4-5us of sustained activity so the HAM
            # clock gate flips to 8/8 (2.4 GHz) before real work; the burst
            # overlaps the first 1 MiB input DMA, so it adds ~no latency.
            p_warm = p1pool.tile([128, 128], F32, tag="p1")
            for _ in range(32):
                nc.tensor.matmul(p_warm[:], lhsT=rhs, rhs=rhs,
                                 start=True, stop=True)

            for g in range(IMGS // ipg):           # image groups
                for hc in range(HC):               # 2 h-chunks
                    hsl = slice(hc * 128, hc * 128 + 128)
                    isl = slice(g * ipg, (g + 1) * ipg)

                    xt = xpool.tile([128, ipg, W], idt)
                    nc.sync.dma_start(
                        out=xt[:],
                        in_=xi[isl, hsl, :].rearrange("i h w -> h i w"),
                    )
                    mt = mpool.tile([128, ipg, W], idt)
                    ot = opool.tile([128, ipg, W], odt)

                    # Subtiles of 128 cols, in quads of 4; each quad's
                    # 4 matmul results fill one PSUM bank [128, 512] so the
                    # PSUM->SBUF copy is one large op instead of four small.
                    fl_x = xt[:].rearrange("p i w -> p (i w)")
                    fl_m = mt[:].rearrange("p i w -> p (i w)")
                    # [128, nsub, 128] views for quad-granular copy dests
                    m4 = mt[:].rearrange("p i (k n) -> p (i k) n", n=128)
                    o4 = ot[:].rearrange("p i (k n) -> p (i k) n", n=128)
                    for q in range(nsub // 4):     # quads of 4 subtiles
                        p1 = p1pool.tile([128, 4, 128], F32, tag="p1")
                        for j in range(4):
                            s = 4 * q + j
                            lhs1 = fl_x[:, 128 * s:128 * s + 128]
                            nc.tensor.matmul(p1[:, j, :], lhsT=lhs1, rhs=rhs,
                                             start=True, stop=True)
                        ssl = slice(4 * q, 4 * q + 4)
                        nc.vector.tensor_copy(m4[:, ssl, :], p1[:, :, :])
                        p2 = p2pool.tile([128, 4, 128], F32, tag="p2")
                        for j in range(4):
                            s = 4 * q + j
                            lhs2 = fl_m[:, 128 * s:128 * s + 128]
                            nc.tensor.matmul(p2[:, j, :], lhsT=lhs2, rhs=rhs,
                                             start=True, stop=True)
                        nc.scalar.copy(o4[:, ssl, :], p2[:, :, :])

                    nc.sync.dma_start(
                        out=oi[isl, hsl, :].rearrange("i h w -> h i w"),
                        in_=ot[:],
                    )
    nc.compile()
    return nc


def _make_r(D):
    return np.ascontiguousarray(
        np.kron(np.eye(32, dtype=np.float32), D.T.astype(np.float32)))


def run(x, D, trace=False, mode=MODE):
    fp16 = mode == "fp16"
    ndt = np.float16 if fp16 else np.float32
    x = np.asarray(x, dtype=np.float32)
    D = np.asarray(D, dtype=np.float32)
    assert x.shape == (B_FULL, C, H, W), x.shape
    r = _make_r(D).astype(ndt)
    xc = np.ascontiguousarray(x.astype(ndt))

    nc = _build_module(mode)
    in_maps = [
        {"x": np.ascontiguousarray(xc[i * B_CORE:(i + 1) * B_CORE]), "r": r}
        for i in range(N_CORES)
    ]
    res = run_bass_kernel_spmd(nc, in_maps, core_ids=list(range(N_CORES)),
                               trace=trace)
    out = np.concatenate([res.results[i]["out"] for i in range(N_CORES)],
                         axis=0)
    return out.astype(np.float32, copy=False), res.exec_time_ns


def kernel(**inputs):
    out, _ = run(inputs["x"], inputs["D"], trace=False)
    return out


# revision 4
# speedup vs baseline: 1.0684x; 1.0684x over previous
"""Trainium2 Bass kernel: 4x4-block 2D DCT over x[16, 64, 256, 256] fp32.

Math: for each 4x4 block B of each 256x256 image, out = D @ B @ D^T.
With R = kron(I_32, D^T) (128x128 block-diagonal), a [128(h), 128(w)] tile X
satisfies:  P1 = X^T @ R   (H-pass, transposed)
            P2 = P1^T @ R  (W-pass, final orientation [h', w'])
Both are single PE matmuls (out = lhsT.T @ rhs with lhsT = data, rhs = R),
so the per-pass transpose comes free from the matmul semantics.

Precision: the harness gate is rel_err < 2e-2; fp16 end-to-end measures
6.5e-4 (host-side numpy simulation vs fp64). fp16 halves HBM traffic
(33.5 MB -> per-core floor ~94 us at 358 GB/s) and runs the PE at
1 cyc/row instead of fp32's 4, so the kernel is DMA-bound as intended.

Sharding: pure data parallel - batch dim 16 -> 2 per core across 8 cores.
Per core: 128 images, processed as 16 supertiles (16 images x 1 h-chunk),
each supertile = one 1 MiB DMA in, 32 chained matmul pairs, one 1 MiB out.
"""

import numpy as np

import concourse.bass as bass
import concourse.mybir as mybir
import concourse.tile as tile
from concourse import bacc
from concourse.bass_utils import run_bass_kernel_spmd

N_CORES = 8
B_FULL, C, H, W = 16, 64, 256, 256
B_CORE = B_FULL // N_CORES          # 2 batches per core
IMGS = B_CORE * C                   # 128 images per core
HC = H // 128                       # h-chunks per image (2)
F32 = mybir.dt.float32
F16 = mybir.dt.float16

# "fp16": fp16 in/mid/out, fp32 PSUM accumulate (rel err ~6.5e-4, 2x less
# HBM traffic + 4x faster PE). "fp32": exact fp32 (rel err ~2.6e-7).
MODE = "fp16"


def _build_module(mode=MODE):
    fp16 = mode == "fp16"
    idt = F16 if fp16 else F32
    odt = F16 if fp16 else F32
    ipg = 16 if fp16 else 8                        # images per supertile

    nc = bacc.Bacc("TRN2", target_bir_lowering=False, debug=False,
                   num_devices=N_CORES)
    x_ap = nc.dram_tensor("x", [B_CORE, C, H, W], idt,
                          kind="ExternalInput").ap()
    r_ap = nc.dram_tensor("r", [128, 128], idt, kind="ExternalInput").ap()
    o_ap = nc.dram_tensor("out", [B_CORE, C, H, W], odt,
                          kind="ExternalOutput").ap()

    xi = x_ap.rearrange("b c h w -> (b c) h w")    # [128, 256, 256]
    oi = o_ap.rearrange("b c h w -> (b c) h w")
    nsub = ipg * W // 128                          # 128-col subtiles/supertile

    with tile.TileContext(nc) as tc:
        with (
            tc.tile_pool(name="const", bufs=1) as cpool,
            tc.tile_pool(name="xin", bufs=4) as xpool,
            tc.tile_pool(name="mid", bufs=3) as mpool,
            tc.tile_pool(name="oout", bufs=3) as opool,
            tc.tile_pool(name="ps1", bufs=2, space="PSUM") as p1pool,
            tc.tile_pool(name="ps2", bufs=2, space="PSUM") as p2pool,
        ):
            r_sb = cpool.tile([128, 128], idt)
            nc.sync.dma_start(out=r_sb[:], in_=r_ap[:])
            rhs = r_sb[:, 0:128]

            # Warm-up burst reading only r_sb: the first matmul absorbs the
            # r_sb DMA wait so no later matmul carries two semaphore waits
            # (Matmult supports at most one). The remaining back-to-back
            # matmuls give the PE ~4-5us of sustained activity so the HAM
            # clock gate flips to 8/8 (2.4 GHz) before real work; the burst
            # overlaps the first 1 MiB input DMA, so it adds ~no latency.
            p_warm = p1pool.tile([128, 128], F32, tag="p1")
            for _ in range(32):
                nc.tensor.matmul(p_warm[:], lhsT=rhs, rhs=rhs,
                                 start=True, stop=True)

            for g in range(IMGS // ipg):           # image groups
                for hc in range(HC):               # 2 h-chunks
                    hsl = slice(hc * 128, hc * 128 + 128)
                    isl = slice(g * ipg, (g + 1) * ipg)

                    xt = xpool.tile([128, ipg, W], idt)
                    nc.sync.dma_start(
                        out=xt[:],
                        in_=xi[isl, hsl, :].rearrange("i h w -> h i w"),
                    )
                    mt = mpool.tile([128, ipg, W], idt)
                    ot = opool.tile([128, ipg, W], odt)

                    # Subtiles of 128 cols, in groups of 8; each group's
                    # 8 matmul results fill two PSUM banks [128, 1024] so the
                    # PSUM->SBUF copy is one large op instead of eight small
                    # (amortizes the ~352-cycle ACT / DVE fixed cost).
                    fl_x = xt[:].rearrange("p i w -> p (i w)")
                    fl_m = mt[:].rearrange("p i w -> p (i w)")
                    # [128, nsub, 128] views for group-granular copy dests
                    m8 = mt[:].rearrange("p i (k n) -> p (i k) n", n=128)
                    o8 = ot[:].rearrange("p i (k n) -> p (i k) n", n=128)
                    for q in range(nsub // 8):     # groups of 8 subtiles
                        p1 = p1pool.tile([128, 8, 128], F32, tag="p1")
                        for j in range(8):
                            s = 8 * q + j
                            lhs1 = fl_x[:, 128 * s:128 * s + 128]
                            nc.tensor.matmul(p1[:, j, :], lhsT=lhs1, rhs=rhs,
                                             start=True, stop=True)
                        ssl = slice(8 * q, 8 * q + 8)
                        nc.vector.tensor_copy(m8[:, ssl, :], p1[:, :, :])
                        p2 = p2pool.tile([128, 8, 128], F32, tag="p2")
                        for j in range(8):
                            s = 8 * q + j
                            lhs2 = fl_m[:, 128 * s:128 * s + 128]
                            nc.tensor.matmul(p2[:, j, :], lhsT=lhs2, rhs=rhs,
                                             start=True, stop=True)
                        nc.scalar.copy(o8[:, ssl, :], p2[:, :, :])

                    # Output DMA issued from the ACT engine: HWDGE has two
                    # physical rings (qSPDynamicHW / qActDynamicHW). Keeping
                    # outputs on the ACT ring means compute-gated output DMAs
                    # never sit in FIFO ahead of input prefetch DMAs.
                    nc.scalar.dma_start(
                        out=oi[isl, hsl, :].rearrange("i h w -> h i w"),
                        in_=ot[:],
                    )
    nc.compile()
    return nc


def _make_r(D):
    return np.ascontiguousarray(
        np.kron(np.eye(32, dtype=np.float32), D.T.astype(np.float32)))


def run(x, D, trace=False, mode=MODE):
    fp16 = mode == "fp16"
    ndt = np.float16 if fp16 else np.float32
    x = np.asarray(x, dtype=np.float32)
    D = np.asarray(D, dtype=np.float32)
    assert x.shape == (B_FULL, C, H, W), x.shape
    r = _make_r(D).astype(ndt)
    xc = np.ascontiguousarray(x.astype(ndt))

    nc = _build_module(mode)
    in_maps = [
        {"x": np.ascontiguousarray(xc[i * B_CORE:(i + 1) * B_CORE]), "r": r}
        for i in range(N_CORES)
    ]
    res = run_bass_kernel_spmd(nc, in_maps, core_ids=list(range(N_CORES)),
                               trace=trace)
    out = np.concatenate([res.results[i]["out"] for i in range(N_CORES)],
                         axis=0)
    return out.astype(np.float32, copy=False), res.exec_time_ns


def kernel(**inputs):
    out, _ = run(inputs["x"], inputs["D"], trace=False)
    return out
